# revision 29
# baseline (speedup 1.0000x reference)
"""Trainium2 Bass kernel for nn_FAM_53377853554972 (channel-attention block).

Per-batch module (B=4, C=256, N=16384):
    a   = Wa @ x + ba            # [C, N]
    b   = Wb @ x + bb
    f   = bn(Wm @ x)             # eval-mode BatchNorm
    att = softmax(a @ b^T, axis=1)
    out = feature + beta * (att @ f)

Algebraic restructuring:
    a b^T = Wa G Wb^T + (Wa r) bb^T + ba (Wb r)^T + N ba bb^T
        with G = x x^T  [C, C]  and  r = x 1  [C]
    att @ f = (att diag(s) Wm) @ x + (att t) 1^T
so the only large GEMMs are the Gram G = x x^T and the final y-pass
    y = (I + M^T)^T-style matmul: y = x + M x + u 1^T
computed as a single bf16 matmul with lhsT = M^T + I (identity folded in
during the M^T PSUM evacuation) and the u bias folded into the PSUM
evacuation ops.  The residual therefore never needs a separate
elementwise pass and x is streamed exactly once (bf16).

Sharding: 8 cores = (batch p = core//2) x (N-half h = core%2).  Each core
streams the FULL batch x^T (fp8) and computes the full-N Gram redundantly
(cheaper than any cross-core exchange); it computes/writes y only for its
own N-half.  No collectives.

DMA choreography (the queues round-robin among co-enqueued transfers, so
a transfer's completion time is set by everything sharing its queue):
  - x^T is split into ascending-size slabs, alternating the two HWDGE
    rings (sync + scalar).  Small slabs complete first, so the gram
    matmuls start ~1.5us in; the big tail slabs complete right at the
    stream's end.  Slabs 0-5 are exactly the first N half, so the
    K1/p/q chain for half 0 overlaps the second half's stream.
  - x [C, NP] (bf16, phase-B input) is split: first quarter on the SWDGE
    ring from t=0, rest on the HWDGE rings but gated (via 1-byte copy
    WAR dependencies) behind the x^T stream so it cannot steal gram
    bandwidth.
  - y is written as 8 [128, 2048] bf16 tiles alternating both HWDGE
    rings as phase B produces them.
"""

import sys

import numpy as np

try:
    import concourse.bass as bass  # noqa: F401
except ImportError:  # pragma: no cover
    sys.path.insert(0, "/opt/trn_rl_repo")
    import concourse.bass as bass  # noqa: F401

import ml_dtypes

import concourse.mybir as mybir
import concourse.tile as tile
from concourse import bacc

B, C, N = 4, 256, 16384
NP = N // 2          # points per core (own half for phase B / output)
NCORES = 8
BN_EPS = 1e-5

F32 = mybir.dt.float32
BF16 = mybir.dt.bfloat16
F8 = mybir.dt.float8e4        # TRN E4M3 (matches OCP e4m3 for |x| < 240)

CA = C + 1                    # 257: gram free dim incl. ones column
CAP = 272                     # chunk pitch: DoubleRow ldweights needs the
                              # K-subtile stride to be a multiple of 16
N_CHUNKS = N // 128           # 128 gram chunks of 128 points
# x^T slab sizes (in chunks), alternating sync/scalar queues.  2-4KB
# per-partition descriptors stream ~2x faster than 8KB ones, and a queue
# round-robins among co-enqueued transfers, so slabs are released at
# most two-per-queue via gate copies (slab k waits on slab k-4).
SLAB_CHUNKS = [8, 8, 12, 12, 16, 16, 16, 16, 12, 12]
assert sum(SLAB_CHUNKS) == N_CHUNKS
# slabs 0..5 (72 chunks) form gram half 0; the rest half 1
H0_SLABS = 6

XBW = 2048                    # xb column quarters
N_XB = NP // XBW              # 4
WIN = 1024                    # phase-B window columns
N_WIN = NP // WIN             # 8

# constants blob column layout (bf16)
CST_WAT = 0
CST_WBT = 512
CST_W2 = 1024
CST_ID = 1536
CST_T2 = 1664
CST_COLS = 1666


def build_nc():
    nc = bacc.Bacc("TRN2", target_bir_lowering=False, debug=False,
                   num_devices=NCORES)

    xta_d = nc.dram_tensor("xta", [128, N_CHUNKS * CAP], F8,
                           kind="ExternalInput")
    xb_d = nc.dram_tensor("xb", [128, 2, NP], BF16, kind="ExternalInput")
    cst_d = nc.dram_tensor("cst", [128, CST_COLS], BF16, kind="ExternalInput")
    crow_d = nc.dram_tensor("crow", [1, 3 * C], BF16, kind="ExternalInput")
    y_d = nc.dram_tensor("y", [C, NP], BF16, kind="ExternalOutput")

    with tile.TileContext(nc) as tc:
        with (
            tc.tile_pool(name="const", bufs=1) as const,
            tc.tile_pool(name="xres", bufs=1) as xres,
            tc.tile_pool(name="small", bufs=1) as small,
            tc.tile_pool(name="ysb", bufs=4) as ysb,
        ):
            # ---- warmup tile first: DVE memset, no DMA dependence ----
            wu_sb = const.tile([128, 256], BF16, tag="wu")
            nc.vector.memset(wu_sb[:], 1.0)

            # ---- x^T slab stream (fp8, full batch, gram input) ----
            rings = [nc.sync, nc.scalar]
            xt_sb = []
            off = 0
            for s, ch in enumerate(SLAB_CHUNKS):
                t = xres.tile([128, ch, CAP], F8, tag=f"xt{s}", name=f"xt{s}")
                xt_sb.append(t)
                if s >= 4:
                    # release gate: at most 2 slabs in flight per queue
                    nc.vector.tensor_copy(t[0:1, 0:1, 0:1],
                                          xt_sb[s - 4][0:1, 0:1, 0:1])
                rings[s % 2].dma_start(
                    out=t[:].rearrange("p j c -> p (j c)"),
                    in_=xta_d[:, off:off + ch * CAP])
                off += ch * CAP

            # ---- constants on the SWDGE ring (lands by ~5us) ----
            cst_sb = const.tile([128, CST_COLS], BF16, tag="cst")
            nc.gpsimd.dma_start(out=cst_sb[:], in_=cst_d[:, :])
            crow_sb = small.tile([1, 3 * C], BF16, tag="crow")
            nc.gpsimd.dma_start(out=crow_sb[:], in_=crow_d[:, :])

            xb_sb = [xres.tile([128, 2, XBW], BF16, tag=f"xb{q}",
                               name=f"xb{q}") for q in range(N_XB)]

            # constant views into the blob
            wat_v = cst_sb[:, CST_WAT:CST_WAT + 512].rearrange(
                "p (i c) -> p i c", c=256)
            wbt_v = cst_sb[:, CST_WBT:CST_WBT + 512].rearrange(
                "p (i c) -> p i c", c=256)
            w2_v = cst_sb[:, CST_W2:CST_W2 + 512].rearrange(
                "p (i c) -> p i c", c=256)
            ident_v = cst_sb[:, CST_ID:CST_ID + 128]
            t2_v = cst_sb[:, CST_T2:CST_T2 + 2]

            prow_sb = small.tile([1, C], BF16, tag="prow")
            qrow_sb = small.tile([1, C], BF16, tag="qrow")
            gaug_sb = small.tile([128, 2, 2, CA], BF16, tag="gaug")

            att_t = [small.tile([128, C], BF16, tag=f"att{ob}",
                                name=f"att{ob}") for ob in range(2)]
            attT_t = [small.tile([128, 2, 128], BF16, tag=f"attT{ob}",
                                 name=f"attT{ob}") for ob in range(2)]
            k1_sb = small.tile([128, 2, C], BF16, tag="k1")
            mt_t = [small.tile([128, 2, 128], BF16, tag=f"mt{ob}",
                               name=f"mt{ob}") for ob in range(2)]
            u_t = [small.tile([128, 1], F32, tag=f"u{ob}",
                              name=f"u{ob}") for ob in range(2)]
            ub_t = [small.tile([128, 512], BF16, tag=f"ub{ob}",
                               name=f"ub{ob}") for ob in range(2)]
            zb_sb = small.tile([128, 512], BF16, tag="zb")
            nc.gpsimd.memset(zb_sb[:], 0.0)

            # single PSUM pool, tags reused across non-overlapping lifetimes:
            #   pa: prow -> h0 -> tp0 -> u0      pb: qrow -> h1 -> tp1 -> u1
            #   pc: warmup -> k1 (both halves)   pd: mt (both halves)
            with tc.tile_pool(name="psh", bufs=1, space="PSUM") as psh:
                wu_ps = psh.tile([128, 2, C], F32, tag="pc", name="wups")
                for _ in range(6):
                    nc.tensor.matmul(wu_ps[:, 0, :], lhsT=wu_sb[:, 0:128],
                                     rhs=wu_sb[:], start=True, stop=True)

                prow_ps = psh.tile([1, C], F32, tag="pa", name="prow")
                qrow_ps = psh.tile([1, C], F32, tag="pb", name="qrow")
                k1_ps = psh.tile([128, 2, C], F32, tag="pc", name="k1p")

                def k1pq_half(h, stop):
                    # K1 += G_h @ Wb^T ; p_row += r_h^T Wa^T ; q_row likewise
                    for cb in range(2):
                        for db in range(2):
                            nc.tensor.matmul(
                                k1_ps[:, cb, :],
                                lhsT=gaug_sb[:, h, db, 128 * cb:128 * (cb + 1)],
                                rhs=wbt_v[:, db, :],
                                start=(h + db == 0), stop=(stop and db == 1))
                    for cb in range(2):
                        nc.tensor.matmul(prow_ps[:],
                                         lhsT=gaug_sb[:, h, cb, C:CA],
                                         rhs=wat_v[:, cb, :],
                                         start=(h + cb == 0),
                                         stop=(stop and cb == 1))
                    for cb in range(2):
                        nc.tensor.matmul(qrow_ps[:],
                                         lhsT=gaug_sb[:, h, cb, C:CA],
                                         rhs=wbt_v[:, cb, :],
                                         start=(h + cb == 0),
                                         stop=(stop and cb == 1))

                with tc.tile_pool(name="psg", bufs=1, space="PSUM") as psg:
                    g_ps = [[psg.tile([128, CA], F32, tag=f"g{h}{cj}",
                                      name=f"g{h}{cj}") for cj in range(2)]
                            for h in range(2)]
                    # gram: fp8 DoubleRow, each matmul contracts TWO chunks
                    first = [True, True]
                    for s, ch in enumerate(SLAB_CHUNKS):
                        h = 0 if s < H0_SLABS else 1
                        xtr = xt_sb[s]
                        last_of_h = (s == H0_SLABS - 1
                                     or s == len(SLAB_CHUNKS) - 1)
                        for jp in range(ch // 2):
                            rhs = xtr[:, 2 * jp:2 * jp + 2, 0:CA]
                            for cj in range(2):
                                nc.tensor.matmul(
                                    g_ps[h][cj][:],
                                    lhsT=xtr[:, 2 * jp:2 * jp + 2,
                                             128 * cj:128 * (cj + 1)],
                                    rhs=rhs,
                                    start=first[h],
                                    stop=(last_of_h and jp == ch // 2 - 1),
                                    perf_mode=mybir.MatmulPerfMode.DoubleRow)
                            first[h] = False
                        if s == H0_SLABS - 1:
                            # half-0 evac + K1 chain overlap the tail slabs
                            nc.scalar.activation(
                                out=gaug_sb[:, 0, 0, :], in_=g_ps[0][0][:],
                                func=mybir.ActivationFunctionType.Copy,
                                bias=0.0, scale=1.0)
                            nc.vector.tensor_copy(gaug_sb[:, 0, 1, :],
                                                  g_ps[0][1][:])
                            k1pq_half(0, stop=False)
                            # xb quarters, gated behind each queue's last
                            # slab (and chained) so they cannot steal gram
                            # bandwidth; gates split vector/gpsimd so no
                            # evacuation chain blocks on them.
                            nc.vector.tensor_copy(xb_sb[0][0:1, 0:1, 0:1],
                                                  xt_sb[8][0:1, 0:1, 0:1])
                            nc.vector.tensor_copy(xb_sb[2][0:1, 0:1, 0:1],
                                                  xt_sb[9][0:1, 0:1, 0:1])
                            nc.gpsimd.tensor_copy(xb_sb[1][0:1, 0:1, 0:1],
                                                  xb_sb[0][0:1, 0:1, 0:1])
                            nc.gpsimd.tensor_copy(xb_sb[3][0:1, 0:1, 0:1],
                                                  xb_sb[2][0:1, 0:1, 0:1])
                            nc.sync.dma_start(out=xb_sb[0][:],
                                              in_=xb_d[:, :, 0:XBW])
                            nc.sync.dma_start(out=xb_sb[1][:],
                                              in_=xb_d[:, :, XBW:2 * XBW])
                            nc.scalar.dma_start(
                                out=xb_sb[2][:],
                                in_=xb_d[:, :, 2 * XBW:3 * XBW])
                    nc.scalar.activation(
                        out=gaug_sb[:, 1, 0, :], in_=g_ps[1][0][:],
                        func=mybir.ActivationFunctionType.Copy,
                        bias=0.0, scale=1.0)
                    nc.vector.tensor_copy(gaug_sb[:, 1, 1, :], g_ps[1][1][:])

                # half-1 K1/p/q with inline evacs so each piece evacuates
                # while the PE works on the next
                for cb in range(2):
                    for db in range(2):
                        nc.tensor.matmul(
                            k1_ps[:, cb, :],
                            lhsT=gaug_sb[:, 1, db, 128 * cb:128 * (cb + 1)],
                            rhs=wbt_v[:, db, :],
                            start=False, stop=(db == 1))
                    if cb == 0:
                        nc.scalar.activation(
                            out=k1_sb[:, 0, :], in_=k1_ps[:, 0, :],
                            func=mybir.ActivationFunctionType.Copy,
                            bias=0.0, scale=1.0)
                    else:
                        nc.vector.tensor_copy(k1_sb[:, 1, :], k1_ps[:, 1, :])
                for cb in range(2):
                    nc.tensor.matmul(prow_ps[:],
                                     lhsT=gaug_sb[:, 1, cb, C:CA],
                                     rhs=wat_v[:, cb, :],
                                     start=False, stop=(cb == 1))
                nc.scalar.activation(
                    out=prow_sb[:], in_=prow_ps[:],
                    func=mybir.ActivationFunctionType.Copy, bias=0.0, scale=1.0)
                for cb in range(2):
                    nc.tensor.matmul(qrow_ps[:],
                                     lhsT=gaug_sb[:, 1, cb, C:CA],
                                     rhs=wbt_v[:, cb, :],
                                     start=False, stop=(cb == 1))
                nc.vector.tensor_copy(qrow_sb[:], qrow_ps[:])
                # last xb quarter (scalar queue, after the h1 evac ACTs)
                nc.scalar.dma_start(out=xb_sb[3][:],
                                    in_=xb_d[:, :, 3 * XBW:4 * XBW])

                # H per o-block: 2 main + 3 rank-1 matmuls, one PSUM group
                h_ps = [psh.tile([128, C], F32, tag=("pa", "pb")[ob],
                                 name=f"h{ob}") for ob in range(2)]
                for ob in range(2):
                    for cb in range(2):
                        nc.tensor.matmul(
                            h_ps[ob][:],
                            lhsT=wat_v[:, cb, 128 * ob:128 * (ob + 1)],
                            rhs=k1_sb[:, cb, :],
                            start=(cb == 0), stop=False)
                    nc.tensor.matmul(
                        h_ps[ob][:],
                        lhsT=prow_sb[0:1, 128 * ob:128 * (ob + 1)],
                        rhs=crow_sb[0:1, 2 * C:3 * C],
                        start=False, stop=False)
                    nc.tensor.matmul(
                        h_ps[ob][:],
                        lhsT=crow_sb[0:1, 128 * ob + C:128 * (ob + 1) + C],
                        rhs=crow_sb[0:1, 2 * C:3 * C],
                        start=False, stop=False)
                    nc.tensor.matmul(
                        h_ps[ob][:],
                        lhsT=crow_sb[0:1, 128 * ob:128 * (ob + 1)],
                        rhs=qrow_sb[:],
                        start=False, stop=True)
                    # softmax of this row block (DVE/ACT run ahead of PE)
                    nmax = small.tile([128, 1], F32, tag=f"nmax{ob}",
                                      name=f"nmax{ob}")
                    nc.vector.reduce_max(nmax[:], h_ps[ob][:],
                                         axis=mybir.AxisListType.X,
                                         negate=True)
                    rsum = small.tile([128, 1], F32, tag=f"rsum{ob}",
                                      name=f"rsum{ob}")
                    nc.scalar.activation(
                        out=att_t[ob][:], in_=h_ps[ob][:],
                        func=mybir.ActivationFunctionType.Exp,
                        bias=nmax[:], scale=1.0, accum_out=rsum[:])
                    rinv = small.tile([128, 1], F32, tag=f"rinv{ob}",
                                      name=f"rinv{ob}")
                    nc.vector.reciprocal(rinv[:], rsum[:])
                    nc.vector.tensor_scalar_mul(att_t[ob][:],
                                                att_t[ob][:], rinv[:])

                # per row block: att^T (paired transpose evac), M^T columns
                # (+ identity on the diagonal block), u column.
                mt_ps = psh.tile([128, 2, C], F32, tag="pd", name="mtp")
                for ob in range(2):
                    tp_ps = psh.tile([128, 2, 128], BF16,
                                     tag=("pa", "pb")[ob])
                    for db in range(2):
                        nc.tensor.transpose(
                            tp_ps[:, db, :],
                            att_t[ob][:, 128 * db:128 * (db + 1)],
                            ident_v)
                    if ob == 0:
                        nc.scalar.activation(
                            out=attT_t[ob][:], in_=tp_ps[:],
                            func=mybir.ActivationFunctionType.Copy,
                            bias=0.0, scale=1.0)
                    else:
                        nc.vector.tensor_copy(attT_t[ob][:], tp_ps[:])
                    for eb in range(2):
                        for db in range(2):
                            nc.tensor.matmul(
                                mt_ps[:, eb, 128 * ob:128 * (ob + 1)],
                                lhsT=w2_v[:, db, 128 * eb:128 * (eb + 1)],
                                rhs=attT_t[ob][:, db, :],
                                start=(db == 0), stop=(db == 1))
                    u_ps = psh.tile([128, 1], F32, tag=("pa", "pb")[ob],
                                    name=f"u{ob}")
                    for db in range(2):
                        nc.tensor.matmul(
                            u_ps[:],
                            lhsT=attT_t[ob][:, db, :],
                            rhs=t2_v[:, db:db + 1],
                            start=(db == 0), stop=(db == 1))
                    for eb in range(2):
                        src = mt_ps[:, eb, 128 * ob:128 * (ob + 1)]
                        if eb == ob:
                            # fold the residual identity into M^T
                            nc.vector.tensor_add(mt_t[ob][:, eb, :],
                                                 src, ident_v)
                        else:
                            nc.scalar.activation(
                                out=mt_t[ob][:, eb, :], in_=src,
                                func=mybir.ActivationFunctionType.Copy,
                                bias=0.0, scale=1.0)
                    nc.vector.tensor_copy(u_t[ob][:], u_ps[:])
                    # broadcast u along 512 cols once: the per-window
                    # evacuation can then use tensor_tensor ADD, which
                    # runs ~2x faster than tensor_scalar/ACT-bias
                    nc.vector.tensor_scalar_add(ub_t[ob][:], zb_sb[:],
                                                u_t[ob][:])

                # ---- phase B: y = (I + M^T)' x  (bf16 matmuls, K=256 in
                # two accumulating halves), u added during PSUM
                # evacuation.  The pool lives INSIDE psh, using the four
                # banks psg freed at gram end, so the first matmuls do
                # not wait on a psh pool-close barrier.  cj=0 windows run
                # first so only mt_t[0] gates the start.
                ORDER = [(0, 0), (1, 0), (0, 1), (1, 1)]
                for w in range(2, N_WIN):
                    ORDER += [(w, 0), (w, 1)]
                EVAC = ["v", "v", "v", "s"]  # vector 24 / scalar 8
                with tc.tile_pool(name="psb", bufs=4, space="PSUM") as psb:
                    ys_t, done = {}, {}
                    ei = 0
                    for w, cj in ORDER:
                        pair = w // 2
                        if (pair, cj) not in ys_t:
                            ys_t[(pair, cj)] = ysb.tile(
                                [128, 2048], BF16, tag=f"ys{cj}",
                                name=f"ys{pair}_{cj}")
                        q, off = divmod(WIN * w, XBW)
                        for wi in range(2):
                            o_ps = psb.tile([128, 512], F32, tag="ops")
                            for eb in range(2):
                                nc.tensor.matmul(
                                    o_ps[:],
                                    lhsT=mt_t[cj][:, eb, :],
                                    rhs=xb_sb[q][:, eb, off + 512 * wi:
                                                 off + 512 * (wi + 1)],
                                    start=(eb == 0), stop=(eb == 1))
                            base = 1024 * (w % 2) + 512 * wi
                            dst = ys_t[(pair, cj)][:, base:base + 512]
                            if EVAC[ei % len(EVAC)] == "s":
                                nc.scalar.activation(
                                    out=dst, in_=o_ps[:],
                                    func=mybir.ActivationFunctionType.Identity,
                                    bias=u_t[cj][:], scale=1.0)
                            else:
                                nc.vector.tensor_add(dst, o_ps[:],
                                                     ub_t[cj][:])
                            ei += 1
                        done[(pair, cj)] = done.get((pair, cj), 0) + 1
                        if done[(pair, cj)] == 2:
                            rings[cj].dma_start(
                                out=y_d[128 * cj:128 * (cj + 1),
                                        2048 * pair:2048 * (pair + 1)],
                                in_=ys_t[(pair, cj)][:])

    nc.compile()
    return nc


_NC_CACHE = None
_RUNNER_CACHE = None


def _get_nc():
    global _NC_CACHE
    if _NC_CACHE is None:
        _NC_CACHE = build_nc()
    return _NC_CACHE


def _get_runner():
    """Persistent sharded jit executable (compile once per process)."""
    global _RUNNER_CACHE
    if _RUNNER_CACHE is not None:
        return _RUNNER_CACHE

    import jax
    from jax.sharding import Mesh, PartitionSpec
    from jax.experimental.shard_map import shard_map

    from concourse import bass2jax
    import concourse.mybir as mb

    nc = _get_nc()
    bass2jax.install_neuronx_cc_hook()
    partition_name = (nc.partition_id_tensor.name
                      if nc.partition_id_tensor else None)

    in_names, out_names, out_avals, zero_outs = [], [], [], []
    for alloc in nc.m.functions[0].allocations:
        if not isinstance(alloc, mb.MemoryLocationSet):
            continue
        name = alloc.memorylocations[0].name
        if alloc.kind == "ExternalInput":
            if name != partition_name:
                in_names.append(name)
        elif alloc.kind == "ExternalOutput":
            out_names.append(name)
            shape = tuple(alloc.tensor_shape)
            dtype = mb.dt.np(alloc.dtype)
            out_avals.append(jax.core.ShapedArray(shape, dtype))
            zero_outs.append(np.zeros(shape, dtype))
    n_params = len(in_names)
    n_outs = len(out_avals)
    all_in_names = list(in_names) + list(out_names)
    if partition_name is not None:
        all_in_names.append(partition_name)
    donate = tuple(range(n_params, n_params + n_outs))

    def _body(*args):
        operands = list(args)
        if partition_name is not None:
            operands.append(bass2jax.partition_id_tensor())
        outs = bass2jax._bass_exec_p.bind(
            *operands,
            out_avals=tuple(out_avals),
            in_names=tuple(all_in_names),
            out_names=tuple(out_names),
            lowering_input_output_aliases=(),
            sim_require_finite=True,
            sim_require_nnan=True,
            nc=nc,
        )
        return tuple(outs)

    devices = jax.devices()[:NCORES]
    assert len(devices) == NCORES
    mesh = Mesh(np.asarray(devices), ("core",))
    in_specs = (PartitionSpec("core"),) * (n_params + n_outs)
    out_specs = (PartitionSpec("core"),) * n_outs
    sharded = jax.jit(
        shard_map(_body, mesh=mesh, in_specs=in_specs, out_specs=out_specs,
                  check_rep=False),
        donate_argnums=donate, keep_unused=True)

    def run(in_maps):
        per_core = [[np.asarray(m[name]) for name in in_names] for m in in_maps]
        concat_in = [
            np.concatenate([per_core[c][i] for c in range(NCORES)], axis=0)
            for i in range(n_params)
        ]
        concat_zeros = [
            np.zeros((NCORES * z.shape[0], *z.shape[1:]), z.dtype)
            for z in zero_outs
        ]
        out_arrs = sharded(*concat_in, *concat_zeros)
        return [
            {name: np.asarray(out_arrs[i]).reshape(NCORES, *out_avals[i].shape)[c]
             for i, name in enumerate(out_names)}
            for c in range(NCORES)
        ]

    _RUNNER_CACHE = run
    return run


def make_in_maps(feature, Wa, ba, Wb, bb, Wm, bn_gamma, bn_beta, bn_mean,
                 bn_var, beta):
    feature = np.asarray(feature, dtype=np.float32)
    Wa = np.asarray(Wa, dtype=np.float32)
    ba = np.asarray(ba, dtype=np.float32)
    Wb = np.asarray(Wb, dtype=np.float32)
    bb = np.asarray(bb, dtype=np.float32)
    Wm = np.asarray(Wm, dtype=np.float32)
    bn_gamma = np.asarray(bn_gamma, dtype=np.float32)
    bn_beta = np.asarray(bn_beta, dtype=np.float32)
    bn_mean = np.asarray(bn_mean, dtype=np.float32)
    bn_var = np.asarray(bn_var, dtype=np.float32)
    beta_v = float(np.asarray(beta).reshape(-1)[0])

    wat = np.ascontiguousarray(Wa.T).astype(ml_dtypes.bfloat16)
    wbt = np.ascontiguousarray(Wb.T).astype(ml_dtypes.bfloat16)
    inv = bn_gamma / np.sqrt(bn_var + BN_EPS)
    w2 = (beta_v * inv[:, None] * Wm).astype(ml_dtypes.bfloat16)
    t2 = (beta_v * (bn_beta - bn_mean * inv)).astype(ml_dtypes.bfloat16)

    # constants blob: [wat | wbt | w2] row-block-interleaved, ident, t2
    cst = np.zeros((128, CST_COLS), ml_dtypes.bfloat16)
    for base, m in ((CST_WAT, wat), (CST_WBT, wbt), (CST_W2, w2)):
        cst[:, base:base + 512] = (
            m.reshape(2, 128, 256).transpose(1, 0, 2).reshape(128, 512))
    cst[:, CST_ID:CST_ID + 128] = np.eye(128, dtype=ml_dtypes.bfloat16)
    cst[:, CST_T2:CST_T2 + 2] = t2.reshape(2, 128).T

    crow = np.concatenate([ba, float(N) * ba, bb]).reshape(1, 3 * C).astype(
        ml_dtypes.bfloat16)

    x_full = feature[..., 0]  # [B, C, N]
    xb_full = x_full.astype(ml_dtypes.bfloat16)
    in_maps = []
    xta_cache = {}
    for core in range(NCORES):
        p, h = divmod(core, 2)
        if p not in xta_cache:
            # x^T_aug packed partition-major: partition q holds, for every
            # chunk j, row n = 128*j + q of [x^T | 1 | pad] (CAP cols).
            xta = np.zeros((N, CAP), ml_dtypes.float8_e4m3)
            xta[:, :C] = x_full[p].T.astype(ml_dtypes.float8_e4m3)
            xta[:, C] = 1.0
            xta_cache[p] = np.ascontiguousarray(
                xta.reshape(N_CHUNKS, 128, CAP).transpose(1, 0, 2)
                .reshape(128, N_CHUNKS * CAP))
        xh = xb_full[p, :, NP * h:NP * (h + 1)]  # [C, NP]
        in_maps.append({
            "xta": xta_cache[p],
            "xb": np.ascontiguousarray(
                xh.reshape(2, 128, NP).transpose(1, 0, 2)),
            "cst": cst, "crow": crow,
        })
    return in_maps


def assemble_out(results):
    out = np.empty((B, C, N), np.float32)
    for core in range(NCORES):
        p, h = divmod(core, 2)
        out[p, :, NP * h:NP * (h + 1)] = results[core]["y"].astype(np.float32)
    return out[..., None]


def kernel(**inputs):
    run = _get_runner()
    in_maps = make_in_maps(**inputs)
    return assemble_out(run(in_maps))


def kernel_profiled(**inputs):
    """Like kernel() but with NTFF tracing; returns (output, BassKernelResults)."""
    from concourse.bass_utils import run_bass_kernel_spmd

    nc = _get_nc()
    in_maps = make_in_maps(**inputs)
    res = run_bass_kernel_spmd(nc, in_maps, core_ids=list(range(NCORES)),
                               trace=True)
    return assemble_out(res.results), res


# revision 31
# speedup vs baseline: 1.0161x; 1.0161x over previous
"""Trainium2 Bass kernel for nn_FAM_53377853554972 (channel-attention block).

Per-batch module (B=4, C=256, N=16384):
    a   = Wa @ x + ba            # [C, N]
    b   = Wb @ x + bb
    f   = bn(Wm @ x)             # eval-mode BatchNorm
    att = softmax(a @ b^T, axis=1)
    out = feature + beta * (att @ f)

Algebraic restructuring:
    a b^T = Wa G Wb^T + (Wa r) bb^T + ba (Wb r)^T + N ba bb^T
        with G = x x^T  [C, C]  and  r = x 1  [C]
    att @ f = (att diag(s) Wm) @ x + (att t) 1^T
so the only large GEMMs are the Gram G = x x^T and the final y-pass
    y = (I + M^T)^T-style matmul: y = x + M x + u 1^T
computed as a single bf16 matmul with lhsT = M^T + I (identity folded in
during the M^T PSUM evacuation) and the u bias folded into the PSUM
evacuation ops.  The residual therefore never needs a separate
elementwise pass and x is streamed exactly once (bf16).

Sharding: 8 cores = (batch p = core//2) x (N-half h = core%2).  Each core
streams the FULL batch x^T (fp8) and computes the full-N Gram redundantly
(cheaper than any cross-core exchange); it computes/writes y only for its
own N-half.  No collectives.

DMA choreography (the queues round-robin among co-enqueued transfers, so
a transfer's completion time is set by everything sharing its queue):
  - x^T is split into ascending-size slabs, alternating the two HWDGE
    rings (sync + scalar).  Small slabs complete first, so the gram
    matmuls start ~1.5us in; the big tail slabs complete right at the
    stream's end.  Slabs 0-5 are exactly the first N half, so the
    K1/p/q chain for half 0 overlaps the second half's stream.
  - x [C, NP] (bf16, phase-B input) is split: first quarter on the SWDGE
    ring from t=0, rest on the HWDGE rings but gated (via 1-byte copy
    WAR dependencies) behind the x^T stream so it cannot steal gram
    bandwidth.
  - y is written as 8 [128, 2048] bf16 tiles alternating both HWDGE
    rings as phase B produces them.
"""

import sys

import numpy as np

try:
    import concourse.bass as bass  # noqa: F401
except ImportError:  # pragma: no cover
    sys.path.insert(0, "/opt/trn_rl_repo")
    import concourse.bass as bass  # noqa: F401

import ml_dtypes

import concourse.mybir as mybir
import concourse.tile as tile
from concourse import bacc

B, C, N = 4, 256, 16384
NP = N // 2          # points per core (own half for phase B / output)
NCORES = 8
BN_EPS = 1e-5

F32 = mybir.dt.float32
BF16 = mybir.dt.bfloat16
F8 = mybir.dt.float8e4        # TRN E4M3 (matches OCP e4m3 for |x| < 240)

CA = C + 1                    # 257: gram free dim incl. ones column
CAP = 272                     # chunk pitch: DoubleRow ldweights needs the
                              # K-subtile stride to be a multiple of 16
N_CHUNKS = N // 128           # 128 gram chunks of 128 points
# x^T slab sizes (in chunks), alternating sync/scalar queues.  2-4KB
# per-partition descriptors stream ~2x faster than 8KB ones, and a queue
# round-robins among co-enqueued transfers, so slabs are released at
# most two-per-queue via gate copies (slab k waits on slab k-4).
SLAB_CHUNKS = [8, 8, 12, 12, 16, 16, 16, 16, 12, 12]
assert sum(SLAB_CHUNKS) == N_CHUNKS
# slabs 0..5 (72 chunks) form gram half 0; the rest half 1
H0_SLABS = 6

XBW = 2048                    # xb column quarters
N_XB = NP // XBW              # 4
WIN = 1024                    # phase-B window columns
N_WIN = NP // WIN             # 8

# constants blob column layout (bf16)
CST_WAT = 0
CST_WBT = 512
CST_W2 = 1024
CST_ID = 1536
CST_T2 = 1664
CST_COLS = 1666


def build_nc():
    nc = bacc.Bacc("TRN2", target_bir_lowering=False, debug=False,
                   num_devices=NCORES)

    xta_d = nc.dram_tensor("xta", [128, N_CHUNKS * CAP], F8,
                           kind="ExternalInput")
    xb_d = nc.dram_tensor("xb", [128, 2, NP], BF16, kind="ExternalInput")
    cst_d = nc.dram_tensor("cst", [128, CST_COLS], BF16, kind="ExternalInput")
    crow_d = nc.dram_tensor("crow", [1, 3 * C], BF16, kind="ExternalInput")
    y_d = nc.dram_tensor("y", [C, NP], BF16, kind="ExternalOutput")

    with tile.TileContext(nc) as tc:
        with (
            tc.tile_pool(name="const", bufs=1) as const,
            tc.tile_pool(name="xres", bufs=1) as xres,
            tc.tile_pool(name="small", bufs=1) as small,
            tc.tile_pool(name="ysb", bufs=4) as ysb,
        ):
            # ---- warmup tile first: DVE memset, no DMA dependence ----
            wu_sb = const.tile([128, 256], BF16, tag="wu")
            nc.vector.memset(wu_sb[:], 1.0)

            # ---- x^T slab stream (fp8, full batch, gram input) ----
            rings = [nc.sync, nc.scalar]
            xt_sb = []
            off = 0
            for s, ch in enumerate(SLAB_CHUNKS):
                t = xres.tile([128, ch, CAP], F8, tag=f"xt{s}", name=f"xt{s}")
                xt_sb.append(t)
                if s >= 4:
                    # release gate: at most 2 slabs in flight per queue
                    nc.vector.tensor_copy(t[0:1, 0:1, 0:1],
                                          xt_sb[s - 4][0:1, 0:1, 0:1])
                rings[s % 2].dma_start(
                    out=t[:].rearrange("p j c -> p (j c)"),
                    in_=xta_d[:, off:off + ch * CAP])
                off += ch * CAP

            # ---- constants on the SWDGE ring (lands by ~5us) ----
            cst_sb = const.tile([128, CST_COLS], BF16, tag="cst")
            nc.gpsimd.dma_start(out=cst_sb[:], in_=cst_d[:, :])
            crow_sb = small.tile([1, 3 * C], BF16, tag="crow")
            nc.gpsimd.dma_start(out=crow_sb[:], in_=crow_d[:, :])

            xb_sb = [xres.tile([128, 2, XBW], BF16, tag=f"xb{q}",
                               name=f"xb{q}") for q in range(N_XB)]

            # constant views into the blob
            wat_v = cst_sb[:, CST_WAT:CST_WAT + 512].rearrange(
                "p (i c) -> p i c", c=256)
            wbt_v = cst_sb[:, CST_WBT:CST_WBT + 512].rearrange(
                "p (i c) -> p i c", c=256)
            w2_v = cst_sb[:, CST_W2:CST_W2 + 512].rearrange(
                "p (i c) -> p i c", c=256)
            ident_v = cst_sb[:, CST_ID:CST_ID + 128]
            t2_v = cst_sb[:, CST_T2:CST_T2 + 2]

            prow_sb = small.tile([1, C], BF16, tag="prow")
            qrow_sb = small.tile([1, C], BF16, tag="qrow")
            gaug_sb = small.tile([128, 2, 2, CA], BF16, tag="gaug")

            att_t = [small.tile([128, C], BF16, tag=f"att{ob}",
                                name=f"att{ob}") for ob in range(2)]
            attT_t = [small.tile([128, 2, 128], BF16, tag=f"attT{ob}",
                                 name=f"attT{ob}") for ob in range(2)]
            k1_sb = small.tile([128, 2, C], BF16, tag="k1")
            mt_t = [small.tile([128, 2, 128], BF16, tag=f"mt{ob}",
                               name=f"mt{ob}") for ob in range(2)]
            u_t = [small.tile([128, 1], F32, tag=f"u{ob}",
                              name=f"u{ob}") for ob in range(2)]
            ub_t = [small.tile([128, 512], BF16, tag=f"ub{ob}",
                               name=f"ub{ob}") for ob in range(2)]
            zb_sb = small.tile([128, 512], BF16, tag="zb")
            nc.gpsimd.memset(zb_sb[:], 0.0)

            # single PSUM pool, tags reused across non-overlapping lifetimes:
            #   pa: prow -> h0 -> tp0 -> u0      pb: qrow -> h1 -> tp1 -> u1
            #   pc: warmup -> k1 (both halves)   pd: mt (both halves)
            with tc.tile_pool(name="psh", bufs=1, space="PSUM") as psh:
                wu_ps = psh.tile([128, 2, C], F32, tag="pc", name="wups")
                for _ in range(10):
                    nc.tensor.matmul(wu_ps[:, 0, :], lhsT=wu_sb[:, 0:128],
                                     rhs=wu_sb[:], start=True, stop=True)

                prow_ps = psh.tile([1, C], F32, tag="pa", name="prow")
                qrow_ps = psh.tile([1, C], F32, tag="pb", name="qrow")
                k1_ps = psh.tile([128, 2, C], F32, tag="pc", name="k1p")

                def k1pq_half(h, stop):
                    # K1 += G_h @ Wb^T ; p_row += r_h^T Wa^T ; q_row likewise
                    for cb in range(2):
                        for db in range(2):
                            nc.tensor.matmul(
                                k1_ps[:, cb, :],
                                lhsT=gaug_sb[:, h, db, 128 * cb:128 * (cb + 1)],
                                rhs=wbt_v[:, db, :],
                                start=(h + db == 0), stop=(stop and db == 1))
                    for cb in range(2):
                        nc.tensor.matmul(prow_ps[:],
                                         lhsT=gaug_sb[:, h, cb, C:CA],
                                         rhs=wat_v[:, cb, :],
                                         start=(h + cb == 0),
                                         stop=(stop and cb == 1))
                    for cb in range(2):
                        nc.tensor.matmul(qrow_ps[:],
                                         lhsT=gaug_sb[:, h, cb, C:CA],
                                         rhs=wbt_v[:, cb, :],
                                         start=(h + cb == 0),
                                         stop=(stop and cb == 1))

                with tc.tile_pool(name="psg", bufs=1, space="PSUM") as psg:
                    g_ps = [[psg.tile([128, CA], F32, tag=f"g{h}{cj}",
                                      name=f"g{h}{cj}") for cj in range(2)]
                            for h in range(2)]
                    # gram: fp8 DoubleRow, each matmul contracts TWO chunks
                    first = [True, True]
                    for s, ch in enumerate(SLAB_CHUNKS):
                        h = 0 if s < H0_SLABS else 1
                        xtr = xt_sb[s]
                        last_of_h = (s == H0_SLABS - 1
                                     or s == len(SLAB_CHUNKS) - 1)
                        for jp in range(ch // 2):
                            rhs = xtr[:, 2 * jp:2 * jp + 2, 0:CA]
                            for cj in range(2):
                                nc.tensor.matmul(
                                    g_ps[h][cj][:],
                                    lhsT=xtr[:, 2 * jp:2 * jp + 2,
                                             128 * cj:128 * (cj + 1)],
                                    rhs=rhs,
                                    start=first[h],
                                    stop=(last_of_h and jp == ch // 2 - 1),
                                    perf_mode=mybir.MatmulPerfMode.DoubleRow)
                            first[h] = False
                        if s == H0_SLABS - 1:
                            # half-0 evac + K1 chain overlap the tail slabs
                            nc.scalar.activation(
                                out=gaug_sb[:, 0, 0, :], in_=g_ps[0][0][:],
                                func=mybir.ActivationFunctionType.Copy,
                                bias=0.0, scale=1.0)
                            nc.vector.tensor_copy(gaug_sb[:, 0, 1, :],
                                                  g_ps[0][1][:])
                            k1pq_half(0, stop=False)
                            # xb quarters, gated behind each queue's last
                            # slab (and chained) so they cannot steal gram
                            # bandwidth; gates split vector/gpsimd so no
                            # evacuation chain blocks on them.
                            nc.vector.tensor_copy(xb_sb[0][0:1, 0:1, 0:1],
                                                  xt_sb[8][0:1, 0:1, 0:1])
                            nc.vector.tensor_copy(xb_sb[2][0:1, 0:1, 0:1],
                                                  xt_sb[9][0:1, 0:1, 0:1])
                            nc.gpsimd.tensor_copy(xb_sb[1][0:1, 0:1, 0:1],
                                                  xb_sb[0][0:1, 0:1, 0:1])
                            nc.gpsimd.tensor_copy(xb_sb[3][0:1, 0:1, 0:1],
                                                  xb_sb[2][0:1, 0:1, 0:1])
                            nc.sync.dma_start(out=xb_sb[0][:],
                                              in_=xb_d[:, :, 0:XBW])
                            nc.sync.dma_start(out=xb_sb[1][:],
                                              in_=xb_d[:, :, XBW:2 * XBW])
                            nc.scalar.dma_start(
                                out=xb_sb[2][:],
                                in_=xb_d[:, :, 2 * XBW:3 * XBW])
                    nc.scalar.activation(
                        out=gaug_sb[:, 1, 0, :], in_=g_ps[1][0][:],
                        func=mybir.ActivationFunctionType.Copy,
                        bias=0.0, scale=1.0)
                    nc.vector.tensor_copy(gaug_sb[:, 1, 1, :], g_ps[1][1][:])

                # half-1 K1/p/q with inline evacs so each piece evacuates
                # while the PE works on the next
                for cb in range(2):
                    for db in range(2):
                        nc.tensor.matmul(
                            k1_ps[:, cb, :],
                            lhsT=gaug_sb[:, 1, db, 128 * cb:128 * (cb + 1)],
                            rhs=wbt_v[:, db, :],
                            start=False, stop=(db == 1))
                    if cb == 0:
                        nc.scalar.activation(
                            out=k1_sb[:, 0, :], in_=k1_ps[:, 0, :],
                            func=mybir.ActivationFunctionType.Copy,
                            bias=0.0, scale=1.0)
                    else:
                        nc.vector.tensor_copy(k1_sb[:, 1, :], k1_ps[:, 1, :])
                for cb in range(2):
                    nc.tensor.matmul(prow_ps[:],
                                     lhsT=gaug_sb[:, 1, cb, C:CA],
                                     rhs=wat_v[:, cb, :],
                                     start=False, stop=(cb == 1))
                nc.scalar.activation(
                    out=prow_sb[:], in_=prow_ps[:],
                    func=mybir.ActivationFunctionType.Copy, bias=0.0, scale=1.0)
                for cb in range(2):
                    nc.tensor.matmul(qrow_ps[:],
                                     lhsT=gaug_sb[:, 1, cb, C:CA],
                                     rhs=wbt_v[:, cb, :],
                                     start=False, stop=(cb == 1))
                nc.vector.tensor_copy(qrow_sb[:], qrow_ps[:])
                # last xb quarter (scalar queue, after the h1 evac ACTs)
                nc.scalar.dma_start(out=xb_sb[3][:],
                                    in_=xb_d[:, :, 3 * XBW:4 * XBW])

                # H per o-block: 2 main + 3 rank-1 matmuls, one PSUM group
                h_ps = [psh.tile([128, C], F32, tag=("pa", "pb")[ob],
                                 name=f"h{ob}") for ob in range(2)]
                for ob in range(2):
                    for cb in range(2):
                        nc.tensor.matmul(
                            h_ps[ob][:],
                            lhsT=wat_v[:, cb, 128 * ob:128 * (ob + 1)],
                            rhs=k1_sb[:, cb, :],
                            start=(cb == 0), stop=False)
                    nc.tensor.matmul(
                        h_ps[ob][:],
                        lhsT=prow_sb[0:1, 128 * ob:128 * (ob + 1)],
                        rhs=crow_sb[0:1, 2 * C:3 * C],
                        start=False, stop=False)
                    nc.tensor.matmul(
                        h_ps[ob][:],
                        lhsT=crow_sb[0:1, 128 * ob + C:128 * (ob + 1) + C],
                        rhs=crow_sb[0:1, 2 * C:3 * C],
                        start=False, stop=False)
                    nc.tensor.matmul(
                        h_ps[ob][:],
                        lhsT=crow_sb[0:1, 128 * ob:128 * (ob + 1)],
                        rhs=qrow_sb[:],
                        start=False, stop=True)
                    # softmax of this row block (DVE/ACT run ahead of PE)
                    nmax = small.tile([128, 1], F32, tag=f"nmax{ob}",
                                      name=f"nmax{ob}")
                    nc.vector.reduce_max(nmax[:], h_ps[ob][:],
                                         axis=mybir.AxisListType.X,
                                         negate=True)
                    rsum = small.tile([128, 1], F32, tag=f"rsum{ob}",
                                      name=f"rsum{ob}")
                    nc.scalar.activation(
                        out=att_t[ob][:], in_=h_ps[ob][:],
                        func=mybir.ActivationFunctionType.Exp,
                        bias=nmax[:], scale=1.0, accum_out=rsum[:])
                    rinv = small.tile([128, 1], F32, tag=f"rinv{ob}",
                                      name=f"rinv{ob}")
                    nc.vector.reciprocal(rinv[:], rsum[:])
                    nc.vector.tensor_scalar_mul(att_t[ob][:],
                                                att_t[ob][:], rinv[:])

                # per row block: att^T (paired transpose evac), M^T columns
                # (+ identity on the diagonal block), u column.  The ob=1
                # chain is interleaved with the first phase-B windows so
                # its softmax/evacuation latency hides behind real work.
                mt_ps = psh.tile([128, 2, C], F32, tag="pd", name="mtp")

                def ob_chain(ob):
                    tp_ps = psh.tile([128, 2, 128], BF16,
                                     tag=("pa", "pb")[ob])
                    for db in range(2):
                        nc.tensor.transpose(
                            tp_ps[:, db, :],
                            att_t[ob][:, 128 * db:128 * (db + 1)],
                            ident_v)
                    if ob == 0:
                        nc.scalar.activation(
                            out=attT_t[ob][:], in_=tp_ps[:],
                            func=mybir.ActivationFunctionType.Copy,
                            bias=0.0, scale=1.0)
                    else:
                        nc.vector.tensor_copy(attT_t[ob][:], tp_ps[:])
                    for eb in range(2):
                        for db in range(2):
                            nc.tensor.matmul(
                                mt_ps[:, eb, 128 * ob:128 * (ob + 1)],
                                lhsT=w2_v[:, db, 128 * eb:128 * (eb + 1)],
                                rhs=attT_t[ob][:, db, :],
                                start=(db == 0), stop=(db == 1))
                    u_ps = psh.tile([128, 1], F32, tag=("pa", "pb")[ob],
                                    name=f"u{ob}")
                    for db in range(2):
                        nc.tensor.matmul(
                            u_ps[:],
                            lhsT=attT_t[ob][:, db, :],
                            rhs=t2_v[:, db:db + 1],
                            start=(db == 0), stop=(db == 1))
                    for eb in range(2):
                        src = mt_ps[:, eb, 128 * ob:128 * (ob + 1)]
                        if eb == ob:
                            # fold the residual identity into M^T
                            nc.vector.tensor_add(mt_t[ob][:, eb, :],
                                                 src, ident_v)
                        else:
                            nc.scalar.activation(
                                out=mt_t[ob][:, eb, :], in_=src,
                                func=mybir.ActivationFunctionType.Copy,
                                bias=0.0, scale=1.0)
                    nc.vector.tensor_copy(u_t[ob][:], u_ps[:])
                    # broadcast u along 512 cols once: the per-window
                    # evacuation can then use tensor_tensor ADD
                    nc.vector.tensor_scalar_add(ub_t[ob][:], zb_sb[:],
                                                u_t[ob][:])

                # ---- phase B: y = (I + M^T)' x  (bf16 matmuls, K=256 in
                # two accumulating halves), u added during PSUM
                # evacuation.  The pool lives INSIDE psh, using the four
                # banks psg freed at gram end, so the first matmuls do
                # not wait on a psh pool-close barrier.
                EVAC = ["v", "v", "v", "s"]  # vector 24 / scalar 8
                with tc.tile_pool(name="psb", bufs=4, space="PSUM") as psb:
                    ys_t, done = {}, {}
                    state = {"ei": 0}

                    def emit_window(w, cj):
                        pair = w // 2
                        if (pair, cj) not in ys_t:
                            # the last pair uses two separate 1024-col
                            # tiles so its two writes pipeline
                            shape = 1024 if pair == 3 else 2048
                            ys_t[(pair, cj)] = [
                                ysb.tile([128, shape], BF16, tag=f"ys{cj}",
                                         name=f"ys{pair}_{cj}_0")]
                            if pair == 3:
                                ys_t[(pair, cj)].append(
                                    ysb.tile([128, 1024], BF16,
                                             tag=f"ys{cj}",
                                             name=f"ys{pair}_{cj}_1"))
                        q, off = divmod(WIN * w, XBW)
                        for wi in range(2):
                            o_ps = psb.tile([128, 512], F32, tag="ops")
                            for eb in range(2):
                                nc.tensor.matmul(
                                    o_ps[:],
                                    lhsT=mt_t[cj][:, eb, :],
                                    rhs=xb_sb[q][:, eb, off + 512 * wi:
                                                 off + 512 * (wi + 1)],
                                    start=(eb == 0), stop=(eb == 1))
                            if pair == 3:
                                yt = ys_t[(pair, cj)][w % 2]
                                dst = yt[:, 512 * wi:512 * (wi + 1)]
                            else:
                                yt = ys_t[(pair, cj)][0]
                                base = 1024 * (w % 2) + 512 * wi
                                dst = yt[:, base:base + 512]
                            e = EVAC[state["ei"] % len(EVAC)]
                            if w >= 6:
                                # final windows: both engines in parallel
                                e = "v" if wi == 0 else "s"
                            if e == "s":
                                nc.scalar.activation(
                                    out=dst, in_=o_ps[:],
                                    func=mybir.ActivationFunctionType.Identity,
                                    bias=u_t[cj][:], scale=1.0)
                            else:
                                nc.vector.tensor_add(dst, o_ps[:],
                                                     ub_t[cj][:])
                            state["ei"] += 1
                        done[(pair, cj)] = done.get((pair, cj), 0) + 1
                        if pair == 3:
                            # fire each 1024-col half as soon as it is done
                            nc_ring = rings[cj]
                            half = w % 2
                            nc_ring.dma_start(
                                out=y_d[128 * cj:128 * (cj + 1),
                                        6144 + 1024 * half:
                                        6144 + 1024 * (half + 1)],
                                in_=ys_t[(pair, cj)][half][:])
                        elif done[(pair, cj)] == 2:
                            rings[cj].dma_start(
                                out=y_d[128 * cj:128 * (cj + 1),
                                        2048 * pair:2048 * (pair + 1)],
                                in_=ys_t[(pair, cj)][0][:])

                    ob_chain(0)
                    emit_window(0, 0)
                    emit_window(1, 0)
                    ob_chain(1)
                    for w in range(2, N_WIN):
                        emit_window(w, 0)
                        emit_window(w - 2, 1)
                    emit_window(6, 1)
                    emit_window(7, 1)

    nc.compile()
    return nc


_NC_CACHE = None
_RUNNER_CACHE = None


def _get_nc():
    global _NC_CACHE
    if _NC_CACHE is None:
        _NC_CACHE = build_nc()
    return _NC_CACHE


def _get_runner():
    """Persistent sharded jit executable (compile once per process)."""
    global _RUNNER_CACHE
    if _RUNNER_CACHE is not None:
        return _RUNNER_CACHE

    import jax
    from jax.sharding import Mesh, PartitionSpec
    from jax.experimental.shard_map import shard_map

    from concourse import bass2jax
    import concourse.mybir as mb

    nc = _get_nc()
    bass2jax.install_neuronx_cc_hook()
    partition_name = (nc.partition_id_tensor.name
                      if nc.partition_id_tensor else None)

    in_names, out_names, out_avals, zero_outs = [], [], [], []
    for alloc in nc.m.functions[0].allocations:
        if not isinstance(alloc, mb.MemoryLocationSet):
            continue
        name = alloc.memorylocations[0].name
        if alloc.kind == "ExternalInput":
            if name != partition_name:
                in_names.append(name)
        elif alloc.kind == "ExternalOutput":
            out_names.append(name)
            shape = tuple(alloc.tensor_shape)
            dtype = mb.dt.np(alloc.dtype)
            out_avals.append(jax.core.ShapedArray(shape, dtype))
            zero_outs.append(np.zeros(shape, dtype))
    n_params = len(in_names)
    n_outs = len(out_avals)
    all_in_names = list(in_names) + list(out_names)
    if partition_name is not None:
        all_in_names.append(partition_name)
    donate = tuple(range(n_params, n_params + n_outs))

    def _body(*args):
        operands = list(args)
        if partition_name is not None:
            operands.append(bass2jax.partition_id_tensor())
        outs = bass2jax._bass_exec_p.bind(
            *operands,
            out_avals=tuple(out_avals),
            in_names=tuple(all_in_names),
            out_names=tuple(out_names),
            lowering_input_output_aliases=(),
            sim_require_finite=True,
            sim_require_nnan=True,
            nc=nc,
        )
        return tuple(outs)

    devices = jax.devices()[:NCORES]
    assert len(devices) == NCORES
    mesh = Mesh(np.asarray(devices), ("core",))
    in_specs = (PartitionSpec("core"),) * (n_params + n_outs)
    out_specs = (PartitionSpec("core"),) * n_outs
    sharded = jax.jit(
        shard_map(_body, mesh=mesh, in_specs=in_specs, out_specs=out_specs,
                  check_rep=False),
        donate_argnums=donate, keep_unused=True)

    def run(in_maps):
        per_core = [[np.asarray(m[name]) for name in in_names] for m in in_maps]
        concat_in = [
            np.concatenate([per_core[c][i] for c in range(NCORES)], axis=0)
            for i in range(n_params)
        ]
        concat_zeros = [
            np.zeros((NCORES * z.shape[0], *z.shape[1:]), z.dtype)
            for z in zero_outs
        ]
        out_arrs = sharded(*concat_in, *concat_zeros)
        return [
            {name: np.asarray(out_arrs[i]).reshape(NCORES, *out_avals[i].shape)[c]
             for i, name in enumerate(out_names)}
            for c in range(NCORES)
        ]

    _RUNNER_CACHE = run
    return run


def make_in_maps(feature, Wa, ba, Wb, bb, Wm, bn_gamma, bn_beta, bn_mean,
                 bn_var, beta):
    feature = np.asarray(feature, dtype=np.float32)
    Wa = np.asarray(Wa, dtype=np.float32)
    ba = np.asarray(ba, dtype=np.float32)
    Wb = np.asarray(Wb, dtype=np.float32)
    bb = np.asarray(bb, dtype=np.float32)
    Wm = np.asarray(Wm, dtype=np.float32)
    bn_gamma = np.asarray(bn_gamma, dtype=np.float32)
    bn_beta = np.asarray(bn_beta, dtype=np.float32)
    bn_mean = np.asarray(bn_mean, dtype=np.float32)
    bn_var = np.asarray(bn_var, dtype=np.float32)
    beta_v = float(np.asarray(beta).reshape(-1)[0])

    wat = np.ascontiguousarray(Wa.T).astype(ml_dtypes.bfloat16)
    wbt = np.ascontiguousarray(Wb.T).astype(ml_dtypes.bfloat16)
    inv = bn_gamma / np.sqrt(bn_var + BN_EPS)
    w2 = (beta_v * inv[:, None] * Wm).astype(ml_dtypes.bfloat16)
    t2 = (beta_v * (bn_beta - bn_mean * inv)).astype(ml_dtypes.bfloat16)

    # constants blob: [wat | wbt | w2] row-block-interleaved, ident, t2
    cst = np.zeros((128, CST_COLS), ml_dtypes.bfloat16)
    for base, m in ((CST_WAT, wat), (CST_WBT, wbt), (CST_W2, w2)):
        cst[:, base:base + 512] = (
            m.reshape(2, 128, 256).transpose(1, 0, 2).reshape(128, 512))
    cst[:, CST_ID:CST_ID + 128] = np.eye(128, dtype=ml_dtypes.bfloat16)
    cst[:, CST_T2:CST_T2 + 2] = t2.reshape(2, 128).T

    crow = np.concatenate([ba, float(N) * ba, bb]).reshape(1, 3 * C).astype(
        ml_dtypes.bfloat16)

    x_full = feature[..., 0]  # [B, C, N]
    xb_full = x_full.astype(ml_dtypes.bfloat16)
    in_maps = []
    xta_cache = {}
    for core in range(NCORES):
        p, h = divmod(core, 2)
        if p not in xta_cache:
            # x^T_aug packed partition-major: partition q holds, for every
            # chunk j, row n = 128*j + q of [x^T | 1 | pad] (CAP cols).
            xta = np.zeros((N, CAP), ml_dtypes.float8_e4m3)
            xta[:, :C] = x_full[p].T.astype(ml_dtypes.float8_e4m3)
            xta[:, C] = 1.0
            xta_cache[p] = np.ascontiguousarray(
                xta.reshape(N_CHUNKS, 128, CAP).transpose(1, 0, 2)
                .reshape(128, N_CHUNKS * CAP))
        xh = xb_full[p, :, NP * h:NP * (h + 1)]  # [C, NP]
        in_maps.append({
            "xta": xta_cache[p],
            "xb": np.ascontiguousarray(
                xh.reshape(2, 128, NP).transpose(1, 0, 2)),
            "cst": cst, "crow": crow,
        })
    return in_maps


def assemble_out(results):
    out = np.empty((B, C, N), np.float32)
    for core in range(NCORES):
        p, h = divmod(core, 2)
        out[p, :, NP * h:NP * (h + 1)] = results[core]["y"].astype(np.float32)
    return out[..., None]


def kernel(**inputs):
    run = _get_runner()
    in_maps = make_in_maps(**inputs)
    return assemble_out(run(in_maps))


def kernel_profiled(**inputs):
    """Like kernel() but with NTFF tracing; returns (output, BassKernelResults)."""
    from concourse.bass_utils import run_bass_kernel_spmd

    nc = _get_nc()
    in_maps = make_in_maps(**inputs)
    res = run_bass_kernel_spmd(nc, in_maps, core_ids=list(range(NCORES)),
                               trace=True)
    return assemble_out(res.results), res
